# revision 18
# baseline (speedup 1.0000x reference)
"""Distributed attention kernel for Trainium2 (8 NeuronCores) - self-contained.

Grading entry point:  kernel(**inputs) -> np.ndarray
Takes the FULL inputs of nn_Attention (S=4096, D=4096, H=4 heads, head_dim
E=1024), shards across 8 NeuronCores, runs a Bass/Tile SPMD kernel, returns
the FULL [S, H*E] float32 output.
"""

import sys
for _p in ("/opt/trn_rl_repo",):
    if _p not in sys.path:
        sys.path.insert(0, _p)



























import numpy as np
import ml_dtypes

import concourse.bass as bass
import concourse.tile as tile
from concourse import mybir, bacc
from contextlib import ExitStack

BF = mybir.dt.bfloat16
F8 = mybir.dt.float8e4
F32 = mybir.dt.float32
N_CORES = 8
FP8_E = 2  # leading QK^T e-tiles computed in fp8 DoubleRow (2x PE rate)

S = 4096   # sequence length
D = 4096   # hidden
E = 1024   # head dim
H = 4      # heads
TCH = 512  # projection chunk width (t or sq)
SQB = 512  # attention sq block


def build(S=S, D=D, E=E):
    SQ = S // 2          # queries per core
    n_tch = S // TCH     # 8 kv-projection chunks
    n_d = D // 128       # d-tiles (contraction)
    DH = min(8, n_d)     # d-tiles per stream tile
    n_dh = n_d // DH     # stream tiles per chunk
    n_e = E // 128       # e-tiles
    n_sqc = SQ // TCH    # q projection chunks
    n_sqb = SQ // SQB
    n_t = S // 128       # t-tiles in attention
    # PV chunks covering E v-cols + ones col: the ones chunk (last cols) is
    # computed first so the reciprocal is ready early; the final chunk is
    # narrow so the post-last-matmul tail is short.
    n_pv = 3
    pvw = [512, 384, 129]
    pvo = [0, 512, 896]
    pv_order = [2, 0, 1]
    EP = E + 16          # vv padded width (ones col at E, 32B-aligned t-slices)
    n_cc = max(1, min(8, S // TCH))  # exchange chunks
    CCW = S // n_cc      # t-width per exchange chunk
    scale = 1.0 / float(np.sqrt(E))

    nc = bacc.Bacc(None, target_bir_lowering=False, debug=False)

    hs = nc.declare_dram_parameter("hs", [n_tch, n_dh, 128, DH, TCH], BF,
                                   isOutput=False)
    hqs = nc.declare_dram_parameter("hqs", [n_sqc, n_dh, 128, DH, TCH], BF,
                                    isOutput=False)
    dd0 = DH // 2        # startup d-group width (chunk-0 passes)
    n_g = n_d // dd0     # wo d-groups
    wo = nc.declare_dram_parameter("wo", [n_g, 128, n_e, dd0, 128], BF,
                                   isOutput=False)
    wq = nc.declare_dram_parameter("wq", [n_e, 128, n_d, 128], BF, isOutput=False)
    bo = nc.declare_dram_parameter("bo", [128, n_e], F32, isOutput=False)
    bq = nc.declare_dram_parameter("bq", [128, n_e], F32, isOutput=False)
    out_ext = nc.declare_dram_parameter("out", [SQ, E], F32, isOutput=True)

    with tile.TileContext(nc) as tc, ExitStack() as ctx:
        dram = ctx.enter_context(tc.tile_pool(name="dram", bufs=1, space="DRAM"))
        cc_in = [dram.tile([E, CCW], BF, tag=f"cci{i}", name=f"cc_in{i}")
                 for i in range(n_cc)]
        cc_out = [dram.tile([2 * E, CCW], BF, tag=f"cco{i}", name=f"cc_out{i}")
                  for i in range(n_cc)]

        singles = ctx.enter_context(tc.tile_pool(name="singles", bufs=1))
        bo_sb = singles.tile([128, n_e], F32, tag="bo")
        nc.sync.dma_start(out=bo_sb, in_=bo[:, :])
        bq_sb = singles.tile([128, n_e], F32, tag="bq")
        nc.sync.dma_start(out=bq_sb, in_=bq[:, :])
        ones_sb = singles.tile([128, 1], BF, tag="ones")
        nc.vector.memset(ones_sb, 1.0)

        qT_pool = ctx.enter_context(tc.tile_pool(name="qT", bufs=1))
        qT_sb = qT_pool.tile([128, n_e - FP8_E, SQ], BF, tag="qT")
        qT8_sb = qT_pool.tile([128, FP8_E, SQ], F8, tag="qT8")

        stage_pool = ctx.enter_context(tc.tile_pool(name="stage", bufs=3))
        psum_proj = tc.alloc_tile_pool(name="psum_proj", bufs=8, space="PSUM")

        wq_pool = tc.alloc_tile_pool(name="wqp", bufs=1)
        stream = tc.alloc_tile_pool(name="stream", bufs=5)

        def load_chunk(src, ci):
            hts = []
            for dh in range(n_dh):
                ht = stream.tile([128, DH, TCH], BF, tag="ht", name="ht")
                nc.sync.dma_start(out=ht, in_=src[ci, dh])
                hts.append(ht)
            return hts

        def proj_chunk(src, ci, w_sb, b_sb, consume, hts=None, sub=1):
            """One projection chunk, d-group-major: all n_e psums accumulate
            concurrently (8 PSUM banks) so each h tile is read once and its
            buffer frees immediately -> deep DMA prefetch, no boundary stall.
            `sub` splits each ht tile into sub pieces (startup granularity)."""
            if hts is None:
                hts = load_chunk(src, ci)
            pss = [psum_proj.tile([128, TCH], F32, tag="mm", name=f"ps{e}")
                   for e in range(n_e)]
            dd = DH // sub
            for p in range(n_dh * sub):
                dh, h = divmod(p, sub)
                for e in range(n_e):
                    for i in range(h * dd, (h + 1) * dd):
                        d = dh * DH + i
                        nc.tensor.matmul(
                            pss[e], lhsT=w_sb[:, e, d, :],
                            rhs=hts[dh][:, i, :],
                            start=(d == 0), stop=(d == n_d - 1))
            for e in range(n_e):
                consume(e, pss[e], b_sb)

        wq_sb = wq_pool.tile([128, n_e, n_d, 128], BF, tag="wq")
        with tc.tile_pool(name="wop", bufs=1) as wo_pool:
            wo_sb = wo_pool.tile([128, n_e, n_d, 128], BF, tag="wo")
            # Startup: interleave chunk-0 h pieces with wo d-group loads in
            # exact consumption order so the PE starts after ~1 MB instead of
            # ~5 MB.  Chunk 0 runs d-groups of dd0=4 (sub=2); pass p needs
            # its ht half-tile + wo group p.  wo's DRAM layout is d-grouped
            # so each group is one clean contiguous DMA.
            hts0 = [stream.tile([128, DH, TCH], BF, tag="ht", name="ht")
                    for _ in range(n_dh)]

            def ld_wo(g, e0=0, e1=n_e):
                nc.sync.dma_start(
                    out=wo_sb[:, e0:e1, g * dd0:(g + 1) * dd0, :],
                    in_=wo[g][:, e0:e1])

            def ld_ht0_half(p):
                dh, h = divmod(p, 2)
                nc.sync.dma_start(
                    out=hts0[dh][:, h * dd0:(h + 1) * dd0, :],
                    in_=hs[0, dh][:, h * dd0:(h + 1) * dd0, :])

            # First pieces small: queued DMAs spread across the 16 DMA
            # engines, so more (smaller) leading pieces finish sooner.
            nc.sync.dma_start(out=hts0[0][:, 0:2, :], in_=hs[0, 0][:, 0:2, :])
            ld_wo(0, 0, 2)
            nc.sync.dma_start(out=hts0[0][:, 2:4, :], in_=hs[0, 0][:, 2:4, :])
            ld_wo(0, 2, 4)
            ld_wo(0, 4, 6)
            ld_wo(0, 6, 8)
            ld_wo(1, 0, 4)
            ld_wo(1, 4, 8)
            ld_ht0_half(1)
            nc.sync.dma_start(out=hts0[1], in_=hs[0, 1])
            ld_wo(2)
            ld_wo(3)
            nc.sync.dma_start(out=hts0[2], in_=hs[0, 2])
            ld_wo(4)
            ld_wo(5)
            nc.sync.dma_start(out=hts0[3], in_=hs[0, 3])
            ld_wo(6)
            ld_wo(7)

            # ---- P1: own projection (kT on even cores / vT on odd cores) ----
            for tci in range(n_tch):
                t0 = tci * TCH

                def consume_p1(e, ps, b_sb, t0=t0):
                    st = stage_pool.tile([128, TCH], BF, tag="st", name="st")
                    nc.vector.tensor_scalar(
                        out=st, in0=ps, scalar1=b_sb[:, e:e + 1], scalar2=None,
                        op0=mybir.AluOpType.add)
                    nc.sync.dma_start(
                        out=cc_in[t0 // CCW][e * 128:(e + 1) * 128,
                                             t0 % CCW:t0 % CCW + TCH],
                        in_=st)
                if tci == 0:
                    proj_chunk(hs, 0, wo_sb, bo_sb, consume_p1, hts=hts0,
                               sub=2)
                else:
                    proj_chunk(hs, tci, wo_sb, bo_sb, consume_p1)
                if tci == min(3, n_tch - 1):
                    for e in range(n_e):
                        nc.sync.dma_start(out=wq_sb[:, e], in_=wq[e])
                if (tci + 1) % (CCW // TCH) == 0:
                    i = t0 // CCW
                    nc.gpsimd.collective_compute(
                        "AllGather", mybir.AluOpType.bypass,
                        replica_groups=[[0, 1], [2, 3], [4, 5], [6, 7]],
                        ins=[cc_in[i][:, :]], outs=[cc_out[i][:, :]])

        # kT loads into the space freed by wo; overlap the q projection
        with tc.tile_pool(name="attnk", bufs=1, side="right") as attnk:
            kT_sb = attnk.tile([128, n_e - FP8_E, S], BF, tag="kT")
            kT8_sb = attnk.tile([128, FP8_E, S], F8, tag="kT8")

            # ---- P2: q projection ----
            for sqc in range(n_sqc):
                def consume_p2(e, ps, b_sb, s0=sqc * TCH):
                    # leading e-tiles feed the fp8 DoubleRow score matmul
                    dst = (qT8_sb[:, e, s0:s0 + TCH] if e < FP8_E
                           else qT_sb[:, e - FP8_E, s0:s0 + TCH])
                    nc.vector.tensor_scalar(
                        out=dst, in0=ps,
                        scalar1=b_sb[:, e:e + 1], scalar2=None,
                        op0=mybir.AluOpType.add)
                proj_chunk(hqs, sqc, wq_sb, bq_sb, consume_p2)

            # kT/st8/vv loads go on the Scalar HWDGE queue: the Sync queue
            # carries the buffer-gated hqs stream, whose blocked head would
            # delay these loads past the attention start.
            for i in range(n_cc):
                nc.scalar.dma_start(
                    out=kT_sb[:, :, i * CCW:(i + 1) * CCW],
                    in_=cc_out[i][FP8_E * 128:E, :].rearrange(
                        "(i p) t -> p i t", p=128))
                st8 = stage_pool.tile([128, FP8_E, CCW], BF, tag="st8",
                                      name="st8")
                nc.scalar.dma_start(
                    out=st8,
                    in_=cc_out[i][0:FP8_E * 128, :].rearrange(
                        "(i p) t -> p i t", p=128))
                nc.vector.tensor_scalar(
                    out=kT8_sb[:, :, i * CCW:(i + 1) * CCW], in0=st8,
                    scalar1=0.0, scalar2=None, op0=mybir.AluOpType.add)

            stream.release()
            wq_pool.release()
            psum_proj.release()

            # ---- v loads (xbar transpose-DMA) + attention ----
            with tc.tile_pool(name="attnv", bufs=1) as attnv, \
                 tc.tile_pool(name="pT", bufs=1) as pT_pool, \
                 tc.tile_pool(name="opool", bufs=4) as opool, \
                 tc.tile_pool(name="small", bufs=4) as small, \
                 tc.tile_pool(name="psum_s", bufs=3, space="PSUM") as psum_s, \
                 tc.tile_pool(name="psum_pv", bufs=5, space="PSUM") as psum_pv:
                vv = attnv.tile([128, n_t, EP], BF, tag="vv")
                nc.vector.memset(vv[:, :, E:E + 1], 1.0)
                for j in range(n_t):
                    ti = j * 128
                    nc.scalar.dma_start_transpose(
                        out=vv[:, j, 0:E],
                        in_=cc_out[ti // CCW][E:2 * E,
                                              ti % CCW:ti % CCW + 128])

                for b in range(n_sqb):
                    q0 = b * SQB
                    pT = pT_pool.tile([128, n_t, SQB], BF, tag="pT", name="pT")
                    for t in range(n_t):
                        ps_s = psum_s.tile([128, SQB], F32, tag="s", name="ps_s")
                        nc.tensor.matmul(
                            ps_s, lhsT=kT8_sb[:, :, t * 128:(t + 1) * 128],
                            rhs=qT8_sb[:, :, q0:q0 + SQB],
                            start=True, stop=False,
                            perf_mode=mybir.MatmulPerfMode.DoubleRow)
                        for e in range(FP8_E, n_e):
                            nc.tensor.matmul(
                                ps_s,
                                lhsT=kT_sb[:, e - FP8_E,
                                           t * 128:(t + 1) * 128],
                                rhs=qT_sb[:, e - FP8_E, q0:q0 + SQB],
                                start=False, stop=(e == n_e - 1))
                        nc.scalar.activation(
                            out=pT[:, t, :], in_=ps_s,
                            func=mybir.ActivationFunctionType.Exp, scale=scale)
                    for sub in range(SQB // 128):
                        r0 = q0 + sub * 128
                        # ones-column chunk first: reciprocal is ready early
                        # and each chunk normalizes + stores as it completes,
                        # so the post-last-matmul tail is one chunk, not three.
                        rec = None
                        for c in pv_order:
                            psv = psum_pv.tile([128, max(pvw)], F32, tag="pv",
                                               name="psv")
                            psv = psv[:, 0:pvw[c]]
                            for t in range(n_t):
                                nc.tensor.matmul(
                                    psv,
                                    lhsT=pT[:, t, sub * 128:(sub + 1) * 128],
                                    rhs=vv[:, t, pvo[c]:pvo[c] + pvw[c]],
                                    start=(t == 0), stop=(t == n_t - 1))
                            if rec is None:
                                rec = small.tile([128, 1], F32, tag="rec",
                                                 name="rec")
                                nc.vector.reciprocal(
                                    rec, psv[:, pvw[c] - 1:pvw[c]])
                            w = pvw[c] if c < n_pv - 1 else pvw[c] - 1
                            ot = opool.tile([128, max(pvw)], F32, tag="ot",
                                            name="ot")
                            nc.vector.tensor_scalar(
                                out=ot[:, 0:w], in0=psv[:, 0:w],
                                scalar1=rec, scalar2=None,
                                op0=mybir.AluOpType.mult)
                            nc.sync.dma_start(
                                out=out_ext[r0:r0 + 128, pvo[c]:pvo[c] + w],
                                in_=ot[:, 0:w])

    nc.compile()
    return nc


def _tile_w(w, bf):
    """[D, E] f32 -> [n_e, 128, n_d, 128] bf16 with w_t[e,p,d,c] = w[d*128+p, e*128+c]."""
    Dd, Ee = w.shape
    t = np.asarray(w, np.float32).astype(bf)
    t = t.reshape(Dd // 128, 128, Ee // 128, 128)        # [dt, dp, et, ec]
    return np.ascontiguousarray(t.transpose(2, 1, 0, 3))  # [et, dp, dt, ec]


def _tile_w_g(w, bf, dd0=4):
    """[D, E] f32 -> [n_g, 128, n_e, dd0, 128] bf16 (d-grouped for one-DMA
    startup loads): w_g[g,p,e,j,c] = w[(g*dd0+j)*128+p, e*128+c]."""
    Dd, Ee = w.shape
    t = np.asarray(w, np.float32).astype(bf)
    t = t.reshape(Dd // (dd0 * 128), dd0, 128, Ee // 128, 128)  # [g,j,p,e,c]
    return np.ascontiguousarray(t.transpose(0, 2, 3, 1, 4))     # [g,p,e,j,c]


def _tile_h(hT):
    """hT [D, S] bf16 -> [S//TCH, n_dh, 128, DH, TCH], [c,dh,p,i,s] = hT[dh*DH*128+i*128+p, c*TCH+s]."""
    Dd, Ss = hT.shape
    n_d = Dd // 128
    DH = min(8, n_d)
    t = hT.reshape(n_d // DH, DH, 128, Ss // TCH, TCH)   # [dh, i, p, c, s]
    return np.ascontiguousarray(t.transpose(3, 0, 2, 1, 4))  # [c, dh, p, i, s]


def shard_inputs(h, Wq, bq, Wk, bk, Wv, bv):
    """Host-side prep: transpose/tile/cast per core. All outside HW timing."""
    bf = ml_dtypes.bfloat16
    h = np.asarray(h, dtype=np.float32)
    Ss = h.shape[0]
    hT = h.astype(bf).T                                   # [D, S] view
    hs = _tile_h(np.ascontiguousarray(hT))                # [n_tch, ...]
    n_half = (Ss // 2) // TCH
    wq_t = [_tile_w(np.asarray(Wq[g]), bf) for g in range(H)]
    bq_t = [np.ascontiguousarray(
        np.asarray(bq[g], np.float32).reshape(-1, 128).T) for g in range(H)]
    in_maps = []
    for c in range(N_CORES):
        g, parity = c // 2, c % 2
        wo_np = np.asarray(Wk[g] if parity == 0 else Wv[g], np.float32)
        bo_np = np.asarray(bk[g] if parity == 0 else bv[g], np.float32)
        in_maps.append({
            "hs": hs,
            "hqs": hs[parity * n_half:(parity + 1) * n_half],
            "wo": _tile_w_g(wo_np, bf),
            "wq": wq_t[g],
            "bo": np.ascontiguousarray(bo_np.reshape(-1, 128).T),
            "bq": bq_t[g],
        })
    return in_maps


def assemble_output(results, S=S, E=E):
    out = np.zeros((S, H * E), np.float32)
    SQ = S // 2
    for c in range(N_CORES):
        g, parity = c // 2, c % 2
        out[parity * SQ:(parity + 1) * SQ, g * E:(g + 1) * E] = results[c]["out"]
    return out

_NC = None


def _ensure_profiling_shim():
    """If tracing is requested via env but the antenv NTFF hook is missing,
    install a ctypes-based shim so run_bass_kernel_spmd doesn't crash."""
    try:
        import antenv.axon_hooks  # noqa: F401
        return
    except ImportError:
        pass
    try:
        import types
        from trn_agent_boot.trn_boot import _ntff_profile_via_ctypes
        hook = _ntff_profile_via_ctypes('/opt/axon/libaxon_pjrt.so')
        mod = types.ModuleType("antenv.axon_hooks")
        mod.get_axon_ntff_profile_hook = lambda: hook
        mod.set_axon_ntff_profile_hook = lambda h: None
        sys.modules['antenv.axon_hooks'] = mod
        import antenv
        antenv.axon_hooks = mod
        import concourse.bass_utils as _bu
        _orig = _bu.upload_artifacts

        def _safe_upload(tmpdir):
            try:
                return _orig(tmpdir)
            except Exception:
                return tmpdir
        _bu.upload_artifacts = _safe_upload
    except Exception:
        pass


def kernel(**inputs):
    """Full-input / full-output entry point used by the grading harness."""
    global _NC
    from concourse.bass_utils import run_bass_kernel_spmd
    _ensure_profiling_shim()
    if _NC is None:
        _NC = build()
    in_maps = shard_inputs(
        h=np.asarray(inputs["h"]), Wq=np.asarray(inputs["Wq"]),
        bq=np.asarray(inputs["bq"]), Wk=np.asarray(inputs["Wk"]),
        bk=np.asarray(inputs["bk"]), Wv=np.asarray(inputs["Wv"]),
        bv=np.asarray(inputs["bv"]))
    res = run_bass_kernel_spmd(_NC, in_maps, list(range(N_CORES)))
    return assemble_output(res.results)



# revision 19
# speedup vs baseline: 1.0374x; 1.0374x over previous
"""Distributed attention kernel for Trainium2 (8 NeuronCores) - self-contained.

Grading entry point:  kernel(**inputs) -> np.ndarray
Takes the FULL inputs of nn_Attention (S=4096, D=4096, H=4 heads, head_dim
E=1024), shards across 8 NeuronCores, runs a Bass/Tile SPMD kernel, returns
the FULL [S, H*E] float32 output.
"""

import sys
for _p in ("/opt/trn_rl_repo",):
    if _p not in sys.path:
        sys.path.insert(0, _p)



























import numpy as np
import ml_dtypes

import concourse.bass as bass
import concourse.tile as tile
from concourse import mybir, bacc
from contextlib import ExitStack

BF = mybir.dt.bfloat16
F8 = mybir.dt.float8e4
F32 = mybir.dt.float32
N_CORES = 8
FP8_E = 2  # leading QK^T e-tiles computed in fp8 DoubleRow (2x PE rate)

S = 4096   # sequence length
D = 4096   # hidden
E = 1024   # head dim
H = 4      # heads
TCH = 512  # projection chunk width (t or sq)
SQB = 512  # attention sq block


def build(S=S, D=D, E=E):
    SQ = S // 2          # queries per core
    n_tch = S // TCH     # 8 kv-projection chunks
    n_d = D // 128       # d-tiles (contraction)
    DH = min(8, n_d)     # d-tiles per stream tile
    n_dh = n_d // DH     # stream tiles per chunk
    n_e = E // 128       # e-tiles
    n_sqc = SQ // TCH    # q projection chunks
    n_sqb = SQ // SQB
    n_t = S // 128       # t-tiles in attention
    # PV chunks covering E v-cols + ones col: the ones chunk (last cols) is
    # computed first so the reciprocal is ready early; the final chunk is
    # narrow so the post-last-matmul tail is short.
    n_pv = 3
    pvw = [512, 384, 129]
    pvo = [0, 512, 896]
    pv_order = [2, 0, 1]
    EP = E + 16          # vv padded width (ones col at E, 32B-aligned t-slices)
    n_cc = max(1, min(8, S // TCH))  # exchange chunks
    CCW = S // n_cc      # t-width per exchange chunk
    scale = 1.0 / float(np.sqrt(E))

    nc = bacc.Bacc(None, target_bir_lowering=False, debug=False)

    hs = nc.declare_dram_parameter("hs", [n_tch, n_dh, 128, DH, TCH], BF,
                                   isOutput=False)
    hqs = nc.declare_dram_parameter("hqs", [n_sqc, n_dh, 128, DH, TCH], BF,
                                    isOutput=False)
    dd0 = DH // 2        # startup d-group width (chunk-0 passes)
    n_g = n_d // dd0     # wo d-groups
    wo = nc.declare_dram_parameter("wo", [n_g, 128, n_e, dd0, 128], BF,
                                   isOutput=False)
    wq = nc.declare_dram_parameter("wq", [n_e, 128, n_d, 128], BF, isOutput=False)
    bo = nc.declare_dram_parameter("bo", [128, n_e], F32, isOutput=False)
    bq = nc.declare_dram_parameter("bq", [128, n_e], F32, isOutput=False)
    out_ext = nc.declare_dram_parameter("out", [SQ, E], F32, isOutput=True)

    with tile.TileContext(nc) as tc, ExitStack() as ctx:
        dram = ctx.enter_context(tc.tile_pool(name="dram", bufs=1, space="DRAM"))
        cc_in = [dram.tile([E, CCW], BF, tag=f"cci{i}", name=f"cc_in{i}")
                 for i in range(n_cc)]
        cc_out = [dram.tile([2 * E, CCW], BF, tag=f"cco{i}", name=f"cc_out{i}")
                  for i in range(n_cc)]

        singles = ctx.enter_context(tc.tile_pool(name="singles", bufs=1))
        bo_sb = singles.tile([128, n_e], F32, tag="bo")
        nc.sync.dma_start(out=bo_sb, in_=bo[:, :])
        bq_sb = singles.tile([128, n_e], F32, tag="bq")
        nc.sync.dma_start(out=bq_sb, in_=bq[:, :])
        ones_sb = singles.tile([128, 1], BF, tag="ones")
        nc.vector.memset(ones_sb, 1.0)

        qT_pool = ctx.enter_context(tc.tile_pool(name="qT", bufs=1))
        qT_sb = qT_pool.tile([128, n_e - FP8_E, SQ], BF, tag="qT")
        qT8_sb = qT_pool.tile([128, FP8_E, SQ], F8, tag="qT8")

        stage_pool = ctx.enter_context(tc.tile_pool(name="stage", bufs=3))
        psum_proj = tc.alloc_tile_pool(name="psum_proj", bufs=8, space="PSUM")

        wq_pool = tc.alloc_tile_pool(name="wqp", bufs=1)
        stream = tc.alloc_tile_pool(name="stream", bufs=5)

        def load_chunk(src, ci):
            hts = []
            for dh in range(n_dh):
                ht = stream.tile([128, DH, TCH], BF, tag="ht", name="ht")
                nc.sync.dma_start(out=ht, in_=src[ci, dh])
                hts.append(ht)
            return hts

        def proj_chunk(src, ci, w_sb, b_sb, consume, hts=None, sub=1):
            """One projection chunk, d-group-major: all n_e psums accumulate
            concurrently (8 PSUM banks) so each h tile is read once and its
            buffer frees immediately -> deep DMA prefetch, no boundary stall.
            `sub` splits each ht tile into sub pieces (startup granularity)."""
            if hts is None:
                hts = load_chunk(src, ci)
            pss = [psum_proj.tile([128, TCH], F32, tag="mm", name=f"ps{e}")
                   for e in range(n_e)]
            dd = DH // sub
            for p in range(n_dh * sub):
                dh, h = divmod(p, sub)
                for e in range(n_e):
                    for i in range(h * dd, (h + 1) * dd):
                        d = dh * DH + i
                        nc.tensor.matmul(
                            pss[e], lhsT=w_sb[:, e, d, :],
                            rhs=hts[dh][:, i, :],
                            start=(d == 0), stop=(d == n_d - 1))
            for e in range(n_e):
                consume(e, pss[e], b_sb)

        wq_sb = wq_pool.tile([128, n_e, n_d, 128], BF, tag="wq")
        with tc.tile_pool(name="wop", bufs=1) as wo_pool:
            wo_sb = wo_pool.tile([128, n_e, n_d, 128], BF, tag="wo")
            # Startup: interleave chunk-0 h pieces with wo d-group loads in
            # exact consumption order so the PE starts after ~1 MB instead of
            # ~5 MB.  Chunk 0 runs d-groups of dd0=4 (sub=2); pass p needs
            # its ht half-tile + wo group p.  wo's DRAM layout is d-grouped
            # so each group is one clean contiguous DMA.
            hts0 = [stream.tile([128, DH, TCH], BF, tag="ht", name="ht")
                    for _ in range(n_dh)]

            def ld_wo(g, e0=0, e1=n_e):
                nc.sync.dma_start(
                    out=wo_sb[:, e0:e1, g * dd0:(g + 1) * dd0, :],
                    in_=wo[g][:, e0:e1])

            def ld_ht0_half(p):
                dh, h = divmod(p, 2)
                nc.sync.dma_start(
                    out=hts0[dh][:, h * dd0:(h + 1) * dd0, :],
                    in_=hs[0, dh][:, h * dd0:(h + 1) * dd0, :])

            # First pieces small: queued DMAs spread across the 16 DMA
            # engines, so more (smaller) leading pieces finish sooner.
            nc.sync.dma_start(out=hts0[0][:, 0:2, :], in_=hs[0, 0][:, 0:2, :])
            ld_wo(0, 0, 2)
            nc.sync.dma_start(out=hts0[0][:, 2:4, :], in_=hs[0, 0][:, 2:4, :])
            ld_wo(0, 2, 4)
            ld_wo(0, 4, 6)
            ld_wo(0, 6, 8)
            ld_wo(1, 0, 4)
            ld_wo(1, 4, 8)
            ld_ht0_half(1)
            nc.sync.dma_start(out=hts0[1], in_=hs[0, 1])
            ld_wo(2)
            ld_wo(3)
            nc.sync.dma_start(out=hts0[2], in_=hs[0, 2])
            ld_wo(4)
            ld_wo(5)
            nc.sync.dma_start(out=hts0[3], in_=hs[0, 3])
            ld_wo(6)
            ld_wo(7)

            # ---- P1: own projection (kT on even cores / vT on odd cores) ----
            for tci in range(n_tch):
                t0 = tci * TCH

                def consume_p1(e, ps, b_sb, t0=t0):
                    st = stage_pool.tile([128, TCH], BF, tag="st", name="st")
                    nc.vector.tensor_scalar(
                        out=st, in0=ps, scalar1=b_sb[:, e:e + 1], scalar2=None,
                        op0=mybir.AluOpType.add)
                    nc.sync.dma_start(
                        out=cc_in[t0 // CCW][e * 128:(e + 1) * 128,
                                             t0 % CCW:t0 % CCW + TCH],
                        in_=st)
                if tci == 0:
                    proj_chunk(hs, 0, wo_sb, bo_sb, consume_p1, hts=hts0,
                               sub=2)
                else:
                    proj_chunk(hs, tci, wo_sb, bo_sb, consume_p1)
                if tci == min(3, n_tch - 1):
                    for e in range(n_e):
                        nc.sync.dma_start(out=wq_sb[:, e], in_=wq[e])
                if (tci + 1) % (CCW // TCH) == 0:
                    i = t0 // CCW
                    nc.gpsimd.collective_compute(
                        "AllGather", mybir.AluOpType.bypass,
                        replica_groups=[[0, 1], [2, 3], [4, 5], [6, 7]],
                        ins=[cc_in[i][:, :]], outs=[cc_out[i][:, :]])

        # kT loads into the space freed by wo; overlap the q projection
        with tc.tile_pool(name="attnk", bufs=1, side="right") as attnk:
            kT_sb = attnk.tile([128, n_e - FP8_E, S], BF, tag="kT")
            kT8_sb = attnk.tile([128, FP8_E, S], F8, tag="kT8")

            # ---- P2: q projection ----
            for sqc in range(n_sqc):
                def consume_p2(e, ps, b_sb, s0=sqc * TCH):
                    # leading e-tiles feed the fp8 DoubleRow score matmul
                    dst = (qT8_sb[:, e, s0:s0 + TCH] if e < FP8_E
                           else qT_sb[:, e - FP8_E, s0:s0 + TCH])
                    nc.vector.tensor_scalar(
                        out=dst, in0=ps,
                        scalar1=b_sb[:, e:e + 1], scalar2=None,
                        op0=mybir.AluOpType.add)
                proj_chunk(hqs, sqc, wq_sb, bq_sb, consume_p2)

            # kT/st8/vv loads go on the Scalar HWDGE queue: the Sync queue
            # carries the buffer-gated hqs stream, whose blocked head would
            # delay these loads past the attention start.
            for i in range(n_cc):
                nc.scalar.dma_start(
                    out=kT_sb[:, :, i * CCW:(i + 1) * CCW],
                    in_=cc_out[i][FP8_E * 128:E, :].rearrange(
                        "(i p) t -> p i t", p=128))
                st8 = stage_pool.tile([128, FP8_E, CCW], BF, tag="st8",
                                      name="st8")
                nc.scalar.dma_start(
                    out=st8,
                    in_=cc_out[i][0:FP8_E * 128, :].rearrange(
                        "(i p) t -> p i t", p=128))
                nc.vector.tensor_scalar(
                    out=kT8_sb[:, :, i * CCW:(i + 1) * CCW], in0=st8,
                    scalar1=0.0, scalar2=None, op0=mybir.AluOpType.add)

            stream.release()
            wq_pool.release()
            psum_proj.release()

            # ---- v loads (xbar transpose-DMA) + attention ----
            with tc.tile_pool(name="attnv", bufs=1) as attnv, \
                 tc.tile_pool(name="pT", bufs=1) as pT_pool, \
                 tc.tile_pool(name="opool", bufs=4) as opool, \
                 tc.tile_pool(name="small", bufs=4) as small, \
                 tc.tile_pool(name="psum_s", bufs=3, space="PSUM") as psum_s, \
                 tc.tile_pool(name="psum_pv", bufs=5, space="PSUM") as psum_pv:
                vv = attnv.tile([128, n_t, EP], BF, tag="vv")
                nc.vector.memset(vv[:, :, E:E + 1], 1.0)
                # vv transposes stay on the Sync queue: their ~1.3us issue
                # cost would stall the Scalar engine's exp activations, and
                # the attnv pool's SBUF anti-dep gates them on P2 end anyway.
                for j in range(n_t):
                    ti = j * 128
                    nc.sync.dma_start_transpose(
                        out=vv[:, j, 0:E],
                        in_=cc_out[ti // CCW][E:2 * E,
                                              ti % CCW:ti % CCW + 128])

                for b in range(n_sqb):
                    q0 = b * SQB
                    pT = pT_pool.tile([128, n_t, SQB], BF, tag="pT", name="pT")
                    for t in range(n_t):
                        ps_s = psum_s.tile([128, SQB], F32, tag="s", name="ps_s")
                        nc.tensor.matmul(
                            ps_s, lhsT=kT8_sb[:, :, t * 128:(t + 1) * 128],
                            rhs=qT8_sb[:, :, q0:q0 + SQB],
                            start=True, stop=False,
                            perf_mode=mybir.MatmulPerfMode.DoubleRow)
                        for e in range(FP8_E, n_e):
                            nc.tensor.matmul(
                                ps_s,
                                lhsT=kT_sb[:, e - FP8_E,
                                           t * 128:(t + 1) * 128],
                                rhs=qT_sb[:, e - FP8_E, q0:q0 + SQB],
                                start=False, stop=(e == n_e - 1))
                        nc.scalar.activation(
                            out=pT[:, t, :], in_=ps_s,
                            func=mybir.ActivationFunctionType.Exp, scale=scale)
                    for sub in range(SQB // 128):
                        r0 = q0 + sub * 128
                        # ones-column chunk first: reciprocal is ready early
                        # and each chunk normalizes + stores as it completes,
                        # so the post-last-matmul tail is one chunk, not three.
                        rec = None
                        for c in pv_order:
                            psv = psum_pv.tile([128, max(pvw)], F32, tag="pv",
                                               name="psv")
                            psv = psv[:, 0:pvw[c]]
                            for t in range(n_t):
                                nc.tensor.matmul(
                                    psv,
                                    lhsT=pT[:, t, sub * 128:(sub + 1) * 128],
                                    rhs=vv[:, t, pvo[c]:pvo[c] + pvw[c]],
                                    start=(t == 0), stop=(t == n_t - 1))
                            if rec is None:
                                rec = small.tile([128, 1], F32, tag="rec",
                                                 name="rec")
                                nc.vector.reciprocal(
                                    rec, psv[:, pvw[c] - 1:pvw[c]])
                            w = pvw[c] if c < n_pv - 1 else pvw[c] - 1
                            ot = opool.tile([128, max(pvw)], F32, tag="ot",
                                            name="ot")
                            nc.vector.tensor_scalar(
                                out=ot[:, 0:w], in0=psv[:, 0:w],
                                scalar1=rec, scalar2=None,
                                op0=mybir.AluOpType.mult)
                            nc.sync.dma_start(
                                out=out_ext[r0:r0 + 128, pvo[c]:pvo[c] + w],
                                in_=ot[:, 0:w])

    nc.compile()
    return nc


def _tile_w(w, bf):
    """[D, E] f32 -> [n_e, 128, n_d, 128] bf16 with w_t[e,p,d,c] = w[d*128+p, e*128+c]."""
    Dd, Ee = w.shape
    t = np.asarray(w, np.float32).astype(bf)
    t = t.reshape(Dd // 128, 128, Ee // 128, 128)        # [dt, dp, et, ec]
    return np.ascontiguousarray(t.transpose(2, 1, 0, 3))  # [et, dp, dt, ec]


def _tile_w_g(w, bf, dd0=4):
    """[D, E] f32 -> [n_g, 128, n_e, dd0, 128] bf16 (d-grouped for one-DMA
    startup loads): w_g[g,p,e,j,c] = w[(g*dd0+j)*128+p, e*128+c]."""
    Dd, Ee = w.shape
    t = np.asarray(w, np.float32).astype(bf)
    t = t.reshape(Dd // (dd0 * 128), dd0, 128, Ee // 128, 128)  # [g,j,p,e,c]
    return np.ascontiguousarray(t.transpose(0, 2, 3, 1, 4))     # [g,p,e,j,c]


def _tile_h(hT):
    """hT [D, S] bf16 -> [S//TCH, n_dh, 128, DH, TCH], [c,dh,p,i,s] = hT[dh*DH*128+i*128+p, c*TCH+s]."""
    Dd, Ss = hT.shape
    n_d = Dd // 128
    DH = min(8, n_d)
    t = hT.reshape(n_d // DH, DH, 128, Ss // TCH, TCH)   # [dh, i, p, c, s]
    return np.ascontiguousarray(t.transpose(3, 0, 2, 1, 4))  # [c, dh, p, i, s]


def shard_inputs(h, Wq, bq, Wk, bk, Wv, bv):
    """Host-side prep: transpose/tile/cast per core. All outside HW timing."""
    bf = ml_dtypes.bfloat16
    h = np.asarray(h, dtype=np.float32)
    Ss = h.shape[0]
    hT = h.astype(bf).T                                   # [D, S] view
    hs = _tile_h(np.ascontiguousarray(hT))                # [n_tch, ...]
    n_half = (Ss // 2) // TCH
    wq_t = [_tile_w(np.asarray(Wq[g]), bf) for g in range(H)]
    bq_t = [np.ascontiguousarray(
        np.asarray(bq[g], np.float32).reshape(-1, 128).T) for g in range(H)]
    in_maps = []
    for c in range(N_CORES):
        g, parity = c // 2, c % 2
        wo_np = np.asarray(Wk[g] if parity == 0 else Wv[g], np.float32)
        bo_np = np.asarray(bk[g] if parity == 0 else bv[g], np.float32)
        in_maps.append({
            "hs": hs,
            "hqs": hs[parity * n_half:(parity + 1) * n_half],
            "wo": _tile_w_g(wo_np, bf),
            "wq": wq_t[g],
            "bo": np.ascontiguousarray(bo_np.reshape(-1, 128).T),
            "bq": bq_t[g],
        })
    return in_maps


def assemble_output(results, S=S, E=E):
    out = np.zeros((S, H * E), np.float32)
    SQ = S // 2
    for c in range(N_CORES):
        g, parity = c // 2, c % 2
        out[parity * SQ:(parity + 1) * SQ, g * E:(g + 1) * E] = results[c]["out"]
    return out

_NC = None


def _ensure_profiling_shim():
    """If tracing is requested via env but the antenv NTFF hook is missing,
    install a ctypes-based shim so run_bass_kernel_spmd doesn't crash."""
    try:
        import antenv.axon_hooks  # noqa: F401
        return
    except ImportError:
        pass
    try:
        import types
        from trn_agent_boot.trn_boot import _ntff_profile_via_ctypes
        hook = _ntff_profile_via_ctypes('/opt/axon/libaxon_pjrt.so')
        mod = types.ModuleType("antenv.axon_hooks")
        mod.get_axon_ntff_profile_hook = lambda: hook
        mod.set_axon_ntff_profile_hook = lambda h: None
        sys.modules['antenv.axon_hooks'] = mod
        import antenv
        antenv.axon_hooks = mod
        import concourse.bass_utils as _bu
        _orig = _bu.upload_artifacts

        def _safe_upload(tmpdir):
            try:
                return _orig(tmpdir)
            except Exception:
                return tmpdir
        _bu.upload_artifacts = _safe_upload
    except Exception:
        pass


def kernel(**inputs):
    """Full-input / full-output entry point used by the grading harness."""
    global _NC
    from concourse.bass_utils import run_bass_kernel_spmd
    _ensure_profiling_shim()
    if _NC is None:
        _NC = build()
    in_maps = shard_inputs(
        h=np.asarray(inputs["h"]), Wq=np.asarray(inputs["Wq"]),
        bq=np.asarray(inputs["bq"]), Wk=np.asarray(inputs["Wk"]),
        bk=np.asarray(inputs["bk"]), Wv=np.asarray(inputs["Wv"]),
        bv=np.asarray(inputs["bv"]))
    res = run_bass_kernel_spmd(_NC, in_maps, list(range(N_CORES)))
    return assemble_output(res.results)

